# revision 62
# baseline (speedup 1.0000x reference)
"""Causal multi-head attention on 8 Trainium2 NeuronCores.

Problem: B=2, S=2048, H=1024, NH=16, HD=64, fp32 in/out.
Sharding: tensor-parallel over heads (2 heads/core) + AllToAll to exchange
attention context so every core computes the output projection for its own
512-token slice. The matmul path runs in bf16 (inputs converted on host;
PSUM accumulation stays fp32) — 3.6e-3 rel err vs the 2e-2 budget.

Key design decisions:
  - ALL transposes (x per chunk, Wq/Wk/Wv, Wo) run on the DMA engines via
    dma_start_transpose (bf16 XBAR path, 14ns per 16x128 tile). This frees
    the PE of ~46k transpose cycles and removes the ACT/DVE PSUM->SBUF
    copies that PE-based transposes would need.
  - V is projected directly in natural [token, channel] layout (lhsT=xT
    slice, rhs=wvT), so no V transpose is needed to build the PV operand.
  - Q/K biases are folded into the PSUM->SBUF copy on DVE
    (tensor_scalar_add with a per-partition [128,1] bias column).
  - The ctx AllToAll runs in bf16: collective cost is ~15us fixed +
    bytes/40GBps, so halving payload cuts each AllToAll from 41us to 28us.
  - Head-0 attention for all chunks runs in phase L1 together with QKV and
    the first MOVE=3 chunks of head-1: their exps use L1's idle ACT time,
    shortening the ACT-bound L2 phase so that X0 still hides under it.
  - Next-chunk QKV matmuls are interleaved into attention's exp-wait slots
    via a generator (_Filler), keeping the PE queue dense through L1.
  - A warm-keeper block of no-op rank-1 PE matmuls bridges the X1 window:
    the tensor engine stays continuously busy (and at full p-state) from
    the end of L2 until the h1 ctxa DMA lands, so the output-projection
    matmuls right after execute at peak clock instead of restarting the
    ramp cold. NWARM is sized so the block ends just after ctxa is ready.
  - ctxa loads use batched 3D-AP DMAs: the h0 half (one DMA) is emitted
    before X1 so it only waits on X0 and transfers during the collective;
    the h1 half arrives as two balanced 4-block DMAs so E's first
    accumulation steps overlap the tail of the transfer.

Schedule per core c (heads 2c, 2c+1 = channels 128c..128c+127):
  A.  biases + DMA-transpose Wq (then x chunk 0 split in two H-halves so
      the first projection starts ~2us earlier), Wk, Wv.
  L1. Per 512-token chunk: xT [128, ht, 512] via DMA-transpose (prefetched
      2 ahead), project qT/kT [chan, tok] (+bias on DVE), V natural
      [tok, chan] -> v1 blocks [V_h0 | 1 | V_h1 | 1], head-0 attention
      (and head-1 for chunks < MOVE):
        S^T[k, q] = K^T.T @ Q^T (diagonal k-tiles narrowed),
        P = exp(S^T/8) on ACT -> bf16 (0/1 upper-tri mask on DVE),
        ctx[65, 512] += V1.T @ P   (row 64 = softmax denominator),
        normalize: DVE reciprocal + GPSIMD partition-broadcast + DVE mul.
  X0. AllToAll of head-0 ctx (bf16, hides under L2).
  W.  DMA-transpose Wo -> woT [128, it, 1024] (DMA work during L2).
  L2. Head-1 attention for chunks MOVE..7 (broadcast via PE ones-matmul
      while the X0 collective is in flight), h0-ctxa load, X1, warm-keeper.
  E.  out[t, o] = ctx.T @ WoT + bo (rank-1 f32r bias), PSUM->SBUF copies
      alternating between ACT and DVE so consecutive tiles don't queue,
      DMA out; host concatenates the 8 per-core slices.
"""
import sys

if '/opt/trn_rl_repo' not in sys.path:
    sys.path.insert(0, '/opt/trn_rl_repo')

import numpy as np
import ml_dtypes

import concourse.bacc as bacc
import concourse.bass as bass
import concourse.mybir as mybir
from concourse.tile import TileContext
from concourse.bass_utils import run_bass_kernel_spmd
from concourse.masks import (make_causal_mask, make_identity,
                             make_upper_triangular)

F32 = mybir.dt.float32
F32R = mybir.dt.float32r
BF16 = mybir.dt.bfloat16
FP8 = mybir.dt.float8e4
EXP = mybir.ActivationFunctionType.Exp

B, S, H, NH, HD = 2, 2048, 1024, 16, 64
NC = 8
T = B * S                 # 4096 tokens
TC = 512                  # tokens per chunk
NCHUNK = T // TC          # 8
NTT = T // 128            # 32 token tiles
HT = H // 128             # 8 H-tiles
SCALE = 1.0 / np.sqrt(HD)

_cache = {}

AHEAD = 2
AHEADL2 = 3
MOVE = 3
NWARM = 190
NWARM0 = 6
NWARM1 = 0
NWARMC = 0
STBUFS = 4
MASKMM = False
PVSPLIT = False
WORKBUFS = 2
FILLN = 2
YMOD = 1


class _Filler:
    """Holds a generator of filler instruction groups (next-chunk QKV
    matmuls). Attention calls fill() between its own PE ops so the PE
    queue never drains while waiting on ACT exps — this both closes PE
    gaps and keeps the tensor engine's p-state ramp at full clock."""

    def __init__(self):
        self.it = None

    def fill(self, n=1):
        if self.it is None:
            return
        for _ in range(n):
            try:
                next(self.it)
            except StopIteration:
                self.it = None
                return

    def drain(self):
        if self.it is not None:
            for _ in self.it:
                pass
            self.it = None


def _attention(nc, pc, qpool, qT, kT, v1, ones_r, cmask, ident_b, cmask01, a2a_in, ch, h,
               use_pb=True, filler=None, ahead=None):
    """Head-h causal attention for token chunk ch; writes ctx to a2a_in.

    S-matmuls are emitted AHEAD iterations early so the PE never waits on
    ACT. V1 blocks are [V_h0 | 1 | V_h1 | 1] (width 130): head h uses cols
    [65h : 65h+65] = (V_h | ones), so ctx lands in rows 0:64 and the softmax
    denominator in row 64. Normalization: DVE reciprocal of row 64, GPSIMD
    partition-broadcast (L1) or PE ones-matmul broadcast (L2, while the X0
    collective is in flight), DVE multiply -> bf16 staging -> DMA.
    """
    b, lc = ch // 4, ch % 4
    nkt = 4 * lc + 4
    ctx_ps = qpool.tile([128, 512], F32, tag='ctx', bufs=2, name='ctx')

    def col0(kt):
        s = kt - 4 * lc
        return 128 * s if s >= 0 else 0

    sts = {}

    def emit_s(kt):
        g = 16 * b + kt
        c0 = col0(kt)
        diag = kt - 4 * lc >= 0
        st = qpool.tile([128, 512], F32, tag='st', bufs=STBUFS, name='st')
        nc.tensor.matmul(
            st[:, c0:512],
            kT[64 * h:64 * (h + 1), 128 * g:128 * (g + 1)],
            qT[64 * h:64 * (h + 1), TC * ch + c0:TC * (ch + 1)],
            start=True, stop=not (diag and MASKMM))
        if diag and MASKMM:
            # causal mask accumulated into PSUM on the PE (cmask^T via
            # identity), so exp feeds the PV matmul with no DVE hop
            nc.tensor.matmul(st[:, c0:c0 + 128], cmask[:], ident_b[:],
                             start=False, stop=True)
        sts[kt] = st

    # PV emission plan: diagonal tiles split into an unmasked part
    # (consumed right after exp — no DVE-mask dependency) and a masked
    # 128-col part deferred one iteration, so the mask multiply never sits
    # on the PE's critical path. n_emit[kt] = how many PV matmuls to emit
    # at step kt; the deferred part of kt is emitted at step kt+1 (or at
    # the end for the last kt).
    parts = []     # (kt, lo, hi, delayed_by)
    for kt in range(nkt):
        s = kt - 4 * lc
        c0 = col0(kt)
        if PVSPLIT and s >= 0 and not MASKMM and c0 + 128 < 512:
            parts.append((kt, c0 + 128, 512, 0))
            parts.append((kt, c0, c0 + 128, 1))
        else:
            parts.append((kt, c0, 512, 0))
    nparts = len(parts)
    emis = sorted(range(nparts),
                  key=lambda i: (parts[i][0] + parts[i][3], parts[i][0]))
    sched = {}
    for rank, i in enumerate(emis):
        kt_, lo, hi, d = parts[i]
        sched.setdefault(min(kt_ + d, nkt - 1), []).append((rank, kt_, lo, hi))

    if ahead is None:
        ahead = AHEAD
    ps = {}
    for j in range(min(ahead + 1, nkt)):
        emit_s(j)
    for kt in range(nkt):
        g = 16 * b + kt
        s = kt - 4 * lc
        c0 = col0(kt)
        st = sts.pop(kt)
        p = pc.tile([128, 512], BF16, tag='p', bufs=4, name='p')
        ps[kt] = p
        nc.scalar.activation(p[:, c0:512], st[:, c0:512], EXP, scale=float(SCALE))
        if s >= 0 and not MASKMM:
            nc.vector.tensor_mul(p[:, c0:c0 + 128], p[:, c0:c0 + 128],
                                 cmask01[:])
        if kt + ahead + 1 < nkt:
            emit_s(kt + ahead + 1)
        if filler is not None:
            filler.fill(FILLN)
        for idx, kt_, lo, hi in sched.get(kt, []):
            g_ = 16 * b + kt_
            nc.tensor.matmul(
                ctx_ps[0:65, lo:hi],
                v1[:, 130 * g_ + 65 * h:130 * g_ + 65 * h + 65],
                ps[kt_][:, lo:hi],
                start=(idx == 0), stop=(idx == nparts - 1))
    recip_f = pc.tile([1, 512], F32, tag='recip_f', bufs=2, name='recip_f')
    nc.vector.reciprocal(recip_f[:], ctx_ps[64:65, :])
    if use_pb:
        # GPSIMD broadcast — only safe while no collective occupies Pool
        bc_sb = pc.tile([64, 512], F32, tag='bc_sb', bufs=2, name='bc_sb')
        nc.gpsimd.partition_broadcast(bc_sb[:], recip_f[:])
    else:
        recip_r = pc.tile([1, 512], F32R, tag='recip_r', bufs=2, name='recip_r')
        nc.vector.tensor_copy(recip_r[:], recip_f[:])
        bc = qpool.tile([128, 512], F32, tag='work', bufs=WORKBUFS, name='bc')
        nc.tensor.matmul(bc[0:64, :], ones_r[0:1, 0:64], recip_r[:],
                         start=True, stop=True)
        bc_sb = pc.tile([64, 512], F32, tag='bc_sb', bufs=2, name='bc_sb')
        nc.vector.tensor_copy(bc_sb[:], bc[0:64, :])
    ctx_sb = pc.tile([64, 512], BF16, tag='ctx_sb', bufs=3, name='ctx_sb')
    nc.vector.tensor_mul(ctx_sb[:], ctx_ps[0:64, :], bc_sb[:])
    nc.sync.dma_start(a2a_in[ch, :, :], ctx_sb[:])


def _build(phases='ALWE'):
    key = ('nc', phases)
    if key in _cache:
        return _cache[key]
    nc = bacc.Bacc('TRN2', target_bir_lowering=False, debug=False, num_devices=NC)

    hs_d = nc.dram_tensor('hs', [T, H], BF16, kind='ExternalInput')
    wq_d = nc.dram_tensor('wq', [128, H], BF16, kind='ExternalInput')
    wk_d = nc.dram_tensor('wk', [128, H], BF16, kind='ExternalInput')
    wv_d = nc.dram_tensor('wv', [128, H], BF16, kind='ExternalInput')
    wo_d = nc.dram_tensor('wo', [H, H], BF16, kind='ExternalInput')
    bq_d = nc.dram_tensor('bq', [128, 1], F32, kind='ExternalInput')
    bk_d = nc.dram_tensor('bk', [128, 1], F32, kind='ExternalInput')
    bv_d = nc.dram_tensor('bv', [1, 128], F32, kind='ExternalInput')
    bo_d = nc.dram_tensor('bo', [1, H], F32, kind='ExternalInput')
    out_d = nc.dram_tensor('out', [TC, H], F32, kind='ExternalOutput')

    with TileContext(nc) as tc:
        with tc.tile_pool(name='persist', bufs=1) as pp, \
             tc.tile_pool(name='scr', bufs=1) as sc, \
             tc.tile_pool(name='dram', bufs=1, space='DRAM') as dpool, \
             tc.tile_pool(name='psum', bufs=1, space='PSUM') as qpool:

            def ptile(shape, dt, tag):
                return pp.tile(shape, dt, tag=tag, name=tag)

            cmask = ident_b = None
            if MASKMM:
                cm_f = ptile([128, 128], F32, 'cm_f')
                make_causal_mask(nc, cm_f[:], mask_val=-8e9)
                cmask = ptile([128, 128], BF16, 'cmask')
                nc.vector.tensor_copy(cmask[:], cm_f[:])
                id_f = ptile([128, 128], F32, 'id_f')
                make_identity(nc, id_f[:])
                ident_b = ptile([128, 128], BF16, 'ident_b')
                nc.vector.tensor_copy(ident_b[:], id_f[:])
            ut_f = ptile([128, 128], F32, 'ut_f')
            make_upper_triangular(nc, ut_f[:], val=1.0, diag=True)
            ut01 = ptile([128, 128], BF16, 'ut01')
            nc.vector.tensor_copy(ut01[:], ut_f[:])
            ones_f = ptile([128, 512], F32, 'ones_f')
            nc.vector.memset(ones_f[:], 1.0)
            ones_r = ptile([1, 512], F32R, 'ones_r')
            nc.vector.tensor_copy(ones_r[:], ones_f[0:1, :])

            wqT = ptile([128, HT, 128], BF16, 'wqT')
            wkT = ptile([128, HT, 128], BF16, 'wkT')
            wvT = ptile([128, HT, 128], BF16, 'wvT')
            woT = ptile([128, HT, H], BF16, 'woT')
            qT = ptile([128, T], BF16, 'qT')
            kT = ptile([128, T], BF16, 'kT')
            v1 = ptile([128, NTT * 130], BF16, 'v1')
            a2a_in0 = dpool.tile([NCHUNK, 64, TC], BF16)
            a2a_out0 = dpool.tile([NCHUNK, 64, TC], BF16)
            a2a_in1 = dpool.tile([NCHUNK, 64, TC], BF16)
            a2a_out1 = dpool.tile([NCHUNK, 64, TC], BF16)

            def load_xt(ch):
                xT = sc.tile([128, HT, TC], BF16, tag='xT', bufs=3, name='xT')
                nc.sync.dma_start_transpose(
                    xT[:], hs_d[TC * ch:TC * (ch + 1), :])
                return xT

            # DMA issue order matters (the DMA engines are a serial
            # resource): chunk-0 x first, then the small w transposes, then
            # chunk-1 x, so the first projection starts as early as possible.
            nc.sync.dma_start_transpose(wqT[:], wq_d[:])
            xT0 = sc.tile([128, HT, TC], BF16, tag='xT', bufs=3, name='xT')
            nc.sync.dma_start_transpose(xT0[:, 0:4, :], hs_d[0:TC, 0:512])
            nc.sync.dma_start_transpose(xT0[:, 4:8, :], hs_d[0:TC, 512:1024])
            xts = {0: xT0}
            if 'A' in phases:
                for w_src, w_dst in ((wk_d, wkT), (wv_d, wvT)):
                    nc.sync.dma_start_transpose(w_dst[:], w_src[:])
            bq_col = ptile([128, 1], F32, 'bq_col')
            bk_col = ptile([128, 1], F32, 'bk_col')
            nc.sync.dma_start(bq_col[:], bq_d[:])
            nc.sync.dma_start(bk_col[:], bk_d[:])
            bv_r = ptile([1, 128], F32R, 'bv_r')
            bo_r = ptile([1, H], F32R, 'bo_r')
            for dst, src in ((bv_r, bv_d), (bo_r, bo_d)):
                tmp = pp.tile(list(dst.shape), F32, tag=f'{dst.name}_f', name='btmp')
                nc.sync.dma_start(tmp[:], src[:])
                nc.vector.tensor_copy(dst[:], tmp[:])
            xts[1] = load_xt(1)

            # Startup warm block: ~30 no-op matmuls keep the PE busy from
            # t~0.5us until the first xT/wT DMAs land, so the first
            # projection matmuls are priced at a ramped clock.
            warm0 = qpool.tile([128, 512], F32, tag='ctx', bufs=2,
                               name='warm0')
            for i in range(NWARM0):
                nc.tensor.matmul(warm0[:], ones_r[0:1, 0:128], ones_r[0:1, :],
                                 start=(i == 0), stop=(i == NWARM0 - 1))

            # v1 ones columns (col 64 of each 65-block), one strided memset
            ones_dst = bass.AP(v1.tensor, v1.offset + 64,
                               [list(v1.ap[0]), [65, 2 * NTT]])
            nc.vector.memset(ones_dst, 1.0)

            def proj_ops(ch):
                """QKV projections for chunk ch into qT/kT/v1, as a
                generator yielding every ~2 PE matmuls so attention can
                interleave this work into its exp-wait slots."""
                xT = xts.pop(ch)
                for w_t, b_c, dst in ((wqT, bq_col, qT), (wkT, bk_col, kT)):
                    ps = qpool.tile([128, 512], F32, tag='work', bufs=WORKBUFS, name='work')
                    for ht in range(HT):
                        nc.tensor.matmul(
                            ps[:], w_t[:, ht, :], xT[:, ht, :],
                            start=(ht == 0), stop=(ht == HT - 1))
                        if ht % YMOD == YMOD - 1:
                            yield
                    nc.vector.tensor_scalar_add(
                        dst[:, TC * ch:TC * (ch + 1)], ps[:], b_c[:])
                    yield
                # V in natural [tok, chan] layout: 4 col-slice accum
                # groups in one PSUM bank, rank-1 f32r bias
                ps = qpool.tile([128, 512], F32, tag='work', bufs=WORKBUFS, name='work')
                for tt in range(4):
                    cs = slice(128 * tt, 128 * (tt + 1))
                    for ht in range(HT):
                        nc.tensor.matmul(
                            ps[:, cs], xT[:, ht, cs], wvT[:, ht, :],
                            start=(ht == 0), stop=False)
                        if ht % YMOD == YMOD - 1:
                            yield
                    nc.tensor.matmul(ps[:, cs], ones_r[0:1, 0:128],
                                     bv_r[0:1, :], start=False, stop=True)
                    kt = 4 * ch + tt
                    base = 130 * kt
                    # [V_h0 | gap | V_h1]: one strided copy fills cols
                    # base..base+63 and base+65..base+128
                    dst = bass.AP(v1.tensor, v1.offset + base,
                                  [list(v1.ap[0]), [65, 2], [1, 64]])
                    nc.vector.tensor_copy(
                        dst, ps[:, cs].rearrange('p (g c) -> p g c', g=2))
                    yield

            # ---- L1: QKV + head-0 attention for all chunks, plus the first
            # MOVE chunks of head-1 (their exps fill L1's idle ACT time;
            # this shortens the ACT-bound L2 phase so X0 still hides) ----
            filler = _Filler()
            if 'L' in phases:
                for _ in proj_ops(0):
                    pass
                if NWARM1:
                    # small warm block: fills the PE-idle window while the
                    # chunk-1 xT DMA-transpose is still in flight
                    w1 = qpool.tile([128, 512], F32, tag='ctx', bufs=2,
                                    name='warm1')
                    for i in range(NWARM1):
                        nc.tensor.matmul(w1[:], ones_r[0:1, 0:128],
                                         ones_r[0:1, :],
                                         start=(i == 0),
                                         stop=(i == NWARM1 - 1))
                for ch in range(NCHUNK):
                    if ch + 2 < NCHUNK:
                        xts[ch + 2] = load_xt(ch + 2)
                    if ch + 1 < NCHUNK:
                        filler.it = proj_ops(ch + 1)
                    _attention(nc, sc, qpool, qT, kT, v1, ones_r, cmask, ident_b, ut01,
                               a2a_in0, ch, 0, filler=filler)
                    if ch < MOVE:
                        _attention(nc, sc, qpool, qT, kT, v1, ones_r, cmask, ident_b, ut01,
                                   a2a_in1, ch, 1, filler=filler)
                    filler.drain()

                # ---- X0: AllToAll for head 0 (overlaps L2) ----
                nc.gpsimd.collective_compute(
                    'AllToAll', mybir.AluOpType.bypass,
                    replica_groups=[list(range(NC))],
                    ins=[a2a_in0[:]], outs=[a2a_out0[:]],
                )

            # ---- W: Wo transpose on the DMA engines (runs during L2) ----
            if 'W' in phases:
                nc.sync.dma_start_transpose(woT[:], wo_d[:])

            ctxa = pp.tile([128, NC * TC], BF16, tag='ctxa', name='ctxa')

            # ---- L2: remaining head-1 attention ----
            if 'L' in phases:
                for ch in range(MOVE, NCHUNK):
                    if ch == NCHUNK - 1:
                        # h0 half of ctxa: ONE 3D-AP DMA, emitted before X1
                        # (so it waits only on X0) and before the last
                        # chunk's store dispatch (its SP-SEQ hold then
                        # overlaps the final attention, not the X1 launch).
                        nc.sync.dma_start(
                            ctxa[0:64, :].rearrange('p (i t) -> p i t', i=NC),
                            a2a_out0[:].rearrange('i p t -> p i t'))
                    _attention(nc, sc, qpool, qT, kT, v1, ones_r, cmask, ident_b, ut01,
                               a2a_in1, ch, 1, use_pb=False, ahead=AHEADL2)
                nc.gpsimd.collective_compute(
                    'AllToAll', mybir.AluOpType.bypass,
                    replica_groups=[list(range(NC))],
                    ins=[a2a_in1[:]], outs=[a2a_out1[:]],
                )
                # Warm-keeper: one long PE accumulation of no-op rank-1
                # matmuls into a scratch bank. It has no dependencies, so it
                # runs back-to-back from the moment L2's PE work ends until
                # roughly when X1 + the h1 ctxa DMA complete. This keeps the
                # tensor engine continuously busy (and at full p-state)
                # through the collective, so the E matmuls right after are
                # executed (and priced) at peak clock instead of cold.
                warm = qpool.tile([128, 512], F32, tag='ctx', bufs=2,
                                  name='warm')
                for i in range(NWARM):
                    nc.tensor.matmul(warm[:], ones_r[0:1, 0:128],
                                     ones_r[0:1, :],
                                     start=(i == 0), stop=(i == NWARM - 1))

            # ---- E: output projection for my 512 tokens ----
            if 'E' in phases:
                # two balanced halves: E's it=0..3 accumulation runs while
                # blocks 4..7 are still transferring
                half = NC // 2 * TC
                nc.sync.dma_start(
                    ctxa[64:128, 0:half].rearrange('p (i t) -> p i t',
                                                   i=NC // 2),
                    a2a_out1[0:NC // 2].rearrange('i p t -> p i t'))
                nc.sync.dma_start(
                    ctxa[64:128, half:].rearrange('p (i t) -> p i t',
                                                  i=NC // 2),
                    a2a_out1[NC // 2:].rearrange('i p t -> p i t'))
                for tt in range(4):
                    for oc in range(2):
                        ps = qpool.tile([128, 512], F32, tag='st', bufs=STBUFS, name='st')[:]
                        for it in range(NC):
                            nc.tensor.matmul(
                                ps[:],
                                ctxa[:, TC * it + 128 * tt:TC * it + 128 * (tt + 1)],
                                woT[:, it, 512 * oc:512 * (oc + 1)],
                                start=(it == 0), stop=False)
                        nc.tensor.matmul(ps[:], ones_r[0:1, 0:128],
                                         bo_r[0:1, 512 * oc:512 * (oc + 1)],
                                         start=False, stop=True)
                        o_sb = sc.tile([128, 512], F32, tag='o_sb', bufs=2, name='o_sb')
                        # alternate copy engines so consecutive tiles'
                        # PSUM->SBUF copies don't queue behind each other
                        if (2 * tt + oc) % 2 == 0:
                            nc.scalar.copy(o_sb[:], ps)
                        else:
                            nc.vector.tensor_copy(o_sb[:], ps)
                        nc.sync.dma_start(
                            out_d[128 * tt:128 * (tt + 1),
                                  512 * oc:512 * (oc + 1)], o_sb[:])

    nc.compile()
    _cache[key] = nc
    return nc


def kernel(hidden_states, Wq, bq, Wk, bk, Wv, bv, Wo, bo, **run_kwargs):
    nc = _build()
    bf = ml_dtypes.bfloat16
    hs = np.ascontiguousarray(
        np.asarray(hidden_states, np.float32).reshape(T, H)).astype(bf)
    Wq, Wk, Wv, Wo = (np.asarray(w, np.float32).astype(bf)
                      for w in (Wq, Wk, Wv, Wo))
    bq, bk, bv, bo = (np.asarray(b, np.float32) for b in (bq, bk, bv, bo))
    in_maps = []
    for c in range(NC):
        r = slice(128 * c, 128 * (c + 1))
        in_maps.append({
            'hs': hs,
            'wq': np.ascontiguousarray(Wq[r]),
            'wk': np.ascontiguousarray(Wk[r]),
            'wv': np.ascontiguousarray(Wv[r]),
            'wo': Wo,
            'bq': np.ascontiguousarray(bq[r].reshape(128, 1)),
            'bk': np.ascontiguousarray(bk[r].reshape(128, 1)),
            'bv': np.ascontiguousarray(bv[r].reshape(1, 128)),
            'bo': np.ascontiguousarray(bo.reshape(1, H)),
        })
    res = run_bass_kernel_spmd(nc, in_maps, core_ids=list(range(NC)), **run_kwargs)
    out = np.concatenate([res.results[c]['out'] for c in range(NC)], axis=0)
    kernel.last_results = res
    return out.reshape(B, S, H)


# revision 65
# speedup vs baseline: 1.0000x; 1.0000x over previous
"""Causal multi-head attention on 8 Trainium2 NeuronCores.

Problem: B=2, S=2048, H=1024, NH=16, HD=64, fp32 in/out.
Sharding: tensor-parallel over heads (2 heads/core) + AllToAll to exchange
attention context so every core computes the output projection for its own
512-token slice. The matmul path runs in bf16 (inputs converted on host;
PSUM accumulation stays fp32) — 3.6e-3 rel err vs the 2e-2 budget.

Key design decisions:
  - ALL transposes (x per chunk, Wq/Wk/Wv, Wo) run on the DMA engines via
    dma_start_transpose (bf16 XBAR path, 14ns per 16x128 tile). This frees
    the PE of ~46k transpose cycles and removes the ACT/DVE PSUM->SBUF
    copies that PE-based transposes would need.
  - V is projected directly in natural [token, channel] layout (lhsT=xT
    slice, rhs=wvT), so no V transpose is needed to build the PV operand.
  - Q/K biases are folded into the PSUM->SBUF copy on DVE
    (tensor_scalar_add with a per-partition [128,1] bias column).
  - The ctx AllToAll runs in bf16: collective cost is ~15us fixed +
    bytes/40GBps, so halving payload cuts each AllToAll from 41us to 28us.
  - Head-0 attention for all chunks runs in phase L1 together with QKV and
    the first MOVE=3 chunks of head-1: their exps use L1's idle ACT time,
    shortening the ACT-bound L2 phase so that X0 still hides under it.
  - Next-chunk QKV matmuls are interleaved into attention's exp-wait slots
    via a generator (_Filler), keeping the PE queue dense through L1.
  - A warm-keeper block of no-op rank-1 PE matmuls bridges the X1 window:
    the tensor engine stays continuously busy (and at full p-state) from
    the end of L2 until the h1 ctxa DMA lands, so the output-projection
    matmuls right after execute at peak clock instead of restarting the
    ramp cold. NWARM is sized so the block ends just after ctxa is ready.
  - ctxa loads use batched 3D-AP DMAs: the h0 half (one DMA) is emitted
    before X1 so it only waits on X0 and transfers during the collective;
    the h1 half arrives as two balanced 4-block DMAs so E's first
    accumulation steps overlap the tail of the transfer.

Schedule per core c (heads 2c, 2c+1 = channels 128c..128c+127):
  A.  biases + DMA-transpose Wq (then x chunk 0 split in two H-halves so
      the first projection starts ~2us earlier), Wk, Wv.
  L1. Per 512-token chunk: xT [128, ht, 512] via DMA-transpose (prefetched
      2 ahead), project qT/kT [chan, tok] (+bias on DVE), V natural
      [tok, chan] -> v1 blocks [V_h0 | 1 | V_h1 | 1], head-0 attention
      (and head-1 for chunks < MOVE):
        S^T[k, q] = K^T.T @ Q^T (diagonal k-tiles narrowed),
        P = exp(S^T/8) on ACT -> bf16 (0/1 upper-tri mask on DVE),
        ctx[65, 512] += V1.T @ P   (row 64 = softmax denominator),
        normalize: DVE reciprocal + GPSIMD partition-broadcast + DVE mul.
  X0. AllToAll of head-0 ctx (bf16, hides under L2).
  W.  DMA-transpose Wo -> woT [128, it, 1024] (DMA work during L2).
  L2. Head-1 attention for chunks MOVE..7 (broadcast via PE ones-matmul
      while the X0 collective is in flight), h0-ctxa load, X1, warm-keeper.
  E.  out[t, o] = ctx.T @ WoT + bo (rank-1 f32r bias), PSUM->SBUF copies
      alternating between ACT and DVE so consecutive tiles don't queue,
      DMA out; host concatenates the 8 per-core slices.
"""
import sys

if '/opt/trn_rl_repo' not in sys.path:
    sys.path.insert(0, '/opt/trn_rl_repo')

import numpy as np
import ml_dtypes

import concourse.bacc as bacc
import concourse.bass as bass
import concourse.mybir as mybir
from concourse.tile import TileContext
from concourse.bass_utils import run_bass_kernel_spmd
from concourse.masks import (make_causal_mask, make_identity,
                             make_upper_triangular)

F32 = mybir.dt.float32
F32R = mybir.dt.float32r
BF16 = mybir.dt.bfloat16
FP8 = mybir.dt.float8e4
EXP = mybir.ActivationFunctionType.Exp

B, S, H, NH, HD = 2, 2048, 1024, 16, 64
NC = 8
T = B * S                 # 4096 tokens
TC = 512                  # tokens per chunk
NCHUNK = T // TC          # 8
NTT = T // 128            # 32 token tiles
HT = H // 128             # 8 H-tiles
SCALE = 1.0 / np.sqrt(HD)

_cache = {}

AHEAD = 2
AHEADL2 = 3
MOVE = 3
NWARM = 190
NWARM0 = 6
NWARM1 = 0
NWARMC = 0
STBUFS = 4
MASKMM = False
PVSPLIT = False
WORKBUFS = 2
FILLN = 2
YMOD = 1


class _Filler:
    """Holds a generator of filler instruction groups (next-chunk QKV
    matmuls). Attention calls fill() between its own PE ops so the PE
    queue never drains while waiting on ACT exps — this both closes PE
    gaps and keeps the tensor engine's p-state ramp at full clock."""

    def __init__(self):
        self.it = None

    def fill(self, n=1):
        if self.it is None:
            return
        for _ in range(n):
            try:
                next(self.it)
            except StopIteration:
                self.it = None
                return

    def drain(self):
        if self.it is not None:
            for _ in self.it:
                pass
            self.it = None


def _attention(nc, pc, qpool, qT, kT, v1, ones_r, cmask, ident_b, cmask01, a2a_in, ch, h,
               use_pb=True, filler=None, ahead=None):
    """Head-h causal attention for token chunk ch; writes ctx to a2a_in.

    S-matmuls are emitted AHEAD iterations early so the PE never waits on
    ACT. V1 blocks are [V_h0 | 1 | V_h1 | 1] (width 130): head h uses cols
    [65h : 65h+65] = (V_h | ones), so ctx lands in rows 0:64 and the softmax
    denominator in row 64. Normalization: DVE reciprocal of row 64, GPSIMD
    partition-broadcast (L1) or PE ones-matmul broadcast (L2, while the X0
    collective is in flight), DVE multiply -> bf16 staging -> DMA.
    """
    b, lc = ch // 4, ch % 4
    nkt = 4 * lc + 4
    ctx_ps = qpool.tile([128, 512], F32, tag='ctx', bufs=2, name='ctx')

    def col0(kt):
        s = kt - 4 * lc
        return 128 * s if s >= 0 else 0

    sts = {}

    def emit_s(kt):
        g = 16 * b + kt
        c0 = col0(kt)
        diag = kt - 4 * lc >= 0
        st = qpool.tile([128, 512], F32, tag='st', bufs=STBUFS, name='st')
        nc.tensor.matmul(
            st[:, c0:512],
            kT[64 * h:64 * (h + 1), 128 * g:128 * (g + 1)],
            qT[64 * h:64 * (h + 1), TC * ch + c0:TC * (ch + 1)],
            start=True, stop=not (diag and MASKMM))
        if diag and MASKMM:
            # causal mask accumulated into PSUM on the PE (cmask^T via
            # identity), so exp feeds the PV matmul with no DVE hop
            nc.tensor.matmul(st[:, c0:c0 + 128], cmask[:], ident_b[:],
                             start=False, stop=True)
        sts[kt] = st

    # PV emission plan: diagonal tiles split into an unmasked part
    # (consumed right after exp — no DVE-mask dependency) and a masked
    # 128-col part deferred one iteration, so the mask multiply never sits
    # on the PE's critical path. n_emit[kt] = how many PV matmuls to emit
    # at step kt; the deferred part of kt is emitted at step kt+1 (or at
    # the end for the last kt).
    parts = []     # (kt, lo, hi, delayed_by)
    for kt in range(nkt):
        s = kt - 4 * lc
        c0 = col0(kt)
        if PVSPLIT and s >= 0 and not MASKMM and c0 + 128 < 512:
            parts.append((kt, c0 + 128, 512, 0))
            parts.append((kt, c0, c0 + 128, 1))
        else:
            parts.append((kt, c0, 512, 0))
    nparts = len(parts)
    emis = sorted(range(nparts),
                  key=lambda i: (parts[i][0] + parts[i][3], parts[i][0]))
    sched = {}
    for rank, i in enumerate(emis):
        kt_, lo, hi, d = parts[i]
        sched.setdefault(min(kt_ + d, nkt - 1), []).append((rank, kt_, lo, hi))

    if ahead is None:
        ahead = AHEAD
    ps = {}
    for j in range(min(ahead + 1, nkt)):
        emit_s(j)
    for kt in range(nkt):
        g = 16 * b + kt
        s = kt - 4 * lc
        c0 = col0(kt)
        st = sts.pop(kt)
        p = pc.tile([128, 512], BF16, tag='p', bufs=4, name='p')
        ps[kt] = p
        nc.scalar.activation(p[:, c0:512], st[:, c0:512], EXP, scale=float(SCALE))
        if s >= 0 and not MASKMM:
            nc.vector.tensor_mul(p[:, c0:c0 + 128], p[:, c0:c0 + 128],
                                 cmask01[:])
        if kt + ahead + 1 < nkt:
            emit_s(kt + ahead + 1)
        if filler is not None:
            filler.fill(FILLN)
        for idx, kt_, lo, hi in sched.get(kt, []):
            g_ = 16 * b + kt_
            nc.tensor.matmul(
                ctx_ps[0:65, lo:hi],
                v1[:, 130 * g_ + 65 * h:130 * g_ + 65 * h + 65],
                ps[kt_][:, lo:hi],
                start=(idx == 0), stop=(idx == nparts - 1))
    recip_f = pc.tile([1, 512], F32, tag='recip_f', bufs=2, name='recip_f')
    nc.vector.reciprocal(recip_f[:], ctx_ps[64:65, :])
    if use_pb:
        # GPSIMD broadcast — only safe while no collective occupies Pool
        bc_sb = pc.tile([64, 512], F32, tag='bc_sb', bufs=2, name='bc_sb')
        nc.gpsimd.partition_broadcast(bc_sb[:], recip_f[:])
    else:
        recip_r = pc.tile([1, 512], F32R, tag='recip_r', bufs=2, name='recip_r')
        nc.vector.tensor_copy(recip_r[:], recip_f[:])
        bc = qpool.tile([128, 512], F32, tag='work', bufs=WORKBUFS, name='bc')
        nc.tensor.matmul(bc[0:64, :], ones_r[0:1, 0:64], recip_r[:],
                         start=True, stop=True)
        bc_sb = pc.tile([64, 512], F32, tag='bc_sb', bufs=2, name='bc_sb')
        nc.vector.tensor_copy(bc_sb[:], bc[0:64, :])
    ctx_sb = pc.tile([64, 512], BF16, tag='ctx_sb', bufs=3, name='ctx_sb')
    nc.vector.tensor_mul(ctx_sb[:], ctx_ps[0:64, :], bc_sb[:])
    nc.sync.dma_start(a2a_in[ch, :, :], ctx_sb[:])


def _build(phases='ALWE'):
    key = ('nc', phases)
    if key in _cache:
        return _cache[key]
    nc = bacc.Bacc('TRN2', target_bir_lowering=False, debug=False, num_devices=NC)

    hs_d = nc.dram_tensor('hs', [T, H], BF16, kind='ExternalInput')
    wq_d = nc.dram_tensor('wq', [128, H], BF16, kind='ExternalInput')
    wk_d = nc.dram_tensor('wk', [128, H], BF16, kind='ExternalInput')
    wv_d = nc.dram_tensor('wv', [128, H], BF16, kind='ExternalInput')
    wo_d = nc.dram_tensor('wo', [H, H], BF16, kind='ExternalInput')
    bq_d = nc.dram_tensor('bq', [128, 1], F32, kind='ExternalInput')
    bk_d = nc.dram_tensor('bk', [128, 1], F32, kind='ExternalInput')
    bv_d = nc.dram_tensor('bv', [1, 128], F32, kind='ExternalInput')
    bo_d = nc.dram_tensor('bo', [1, H], F32, kind='ExternalInput')
    out_d = nc.dram_tensor('out', [TC, H], F32, kind='ExternalOutput')

    with TileContext(nc) as tc:
        with tc.tile_pool(name='persist', bufs=1) as pp, \
             tc.tile_pool(name='scr', bufs=1) as sc, \
             tc.tile_pool(name='dram', bufs=1, space='DRAM') as dpool, \
             tc.tile_pool(name='psum', bufs=1, space='PSUM') as qpool:

            def ptile(shape, dt, tag):
                return pp.tile(shape, dt, tag=tag, name=tag)

            cmask = ident_b = None
            if MASKMM:
                cm_f = ptile([128, 128], F32, 'cm_f')
                make_causal_mask(nc, cm_f[:], mask_val=-8e9)
                cmask = ptile([128, 128], BF16, 'cmask')
                nc.vector.tensor_copy(cmask[:], cm_f[:])
                id_f = ptile([128, 128], F32, 'id_f')
                make_identity(nc, id_f[:])
                ident_b = ptile([128, 128], BF16, 'ident_b')
                nc.vector.tensor_copy(ident_b[:], id_f[:])
            ut_f = ptile([128, 128], F32, 'ut_f')
            make_upper_triangular(nc, ut_f[:], val=1.0, diag=True)
            ut01 = ptile([128, 128], BF16, 'ut01')
            nc.vector.tensor_copy(ut01[:], ut_f[:])
            ones_f = ptile([128, 512], F32, 'ones_f')
            nc.vector.memset(ones_f[:], 1.0)
            ones_r = ptile([1, 512], F32R, 'ones_r')
            nc.vector.tensor_copy(ones_r[:], ones_f[0:1, :])

            wqT = ptile([128, HT, 128], BF16, 'wqT')
            wkT = ptile([128, HT, 128], BF16, 'wkT')
            wvT = ptile([128, HT, 128], BF16, 'wvT')
            woT = ptile([128, HT, H], BF16, 'woT')
            qT = ptile([128, T], BF16, 'qT')
            kT = ptile([128, T], BF16, 'kT')
            v1 = ptile([128, NTT * 130], BF16, 'v1')
            a2a_in0 = dpool.tile([NCHUNK, 64, TC], BF16)
            a2a_out0 = dpool.tile([NCHUNK, 64, TC], BF16)
            a2a_in1 = dpool.tile([NCHUNK, 64, TC], BF16)
            a2a_out1 = dpool.tile([NCHUNK, 64, TC], BF16)

            def load_xt(ch):
                xT = sc.tile([128, HT, TC], BF16, tag='xT', bufs=3, name='xT')
                nc.sync.dma_start_transpose(
                    xT[:], hs_d[TC * ch:TC * (ch + 1), :])
                return xT

            # DMA issue order matters (the DMA engines are a serial
            # resource): chunk-0 x first, then the small w transposes, then
            # chunk-1 x, so the first projection starts as early as possible.
            nc.sync.dma_start_transpose(wqT[:], wq_d[:])
            xT0 = sc.tile([128, HT, TC], BF16, tag='xT', bufs=3, name='xT')
            nc.sync.dma_start_transpose(xT0[:, 0:4, :], hs_d[0:TC, 0:512])
            nc.sync.dma_start_transpose(xT0[:, 4:8, :], hs_d[0:TC, 512:1024])
            xts = {0: xT0}
            if 'A' in phases:
                for w_src, w_dst in ((wk_d, wkT), (wv_d, wvT)):
                    nc.sync.dma_start_transpose(w_dst[:], w_src[:])
            bq_col = ptile([128, 1], F32, 'bq_col')
            bk_col = ptile([128, 1], F32, 'bk_col')
            nc.sync.dma_start(bq_col[:], bq_d[:])
            nc.sync.dma_start(bk_col[:], bk_d[:])
            bv_r = ptile([1, 128], F32R, 'bv_r')
            bo_r = ptile([1, H], F32R, 'bo_r')
            for dst, src in ((bv_r, bv_d), (bo_r, bo_d)):
                tmp = pp.tile(list(dst.shape), F32, tag=f'{dst.name}_f', name='btmp')
                nc.sync.dma_start(tmp[:], src[:])
                nc.vector.tensor_copy(dst[:], tmp[:])
            xT1 = sc.tile([128, HT, TC], BF16, tag='xT', bufs=3, name='xT')
            nc.sync.dma_start_transpose(xT1[:, 0:4, :],
                                        hs_d[TC:2 * TC, 0:512])
            nc.sync.dma_start_transpose(xT1[:, 4:8, :],
                                        hs_d[TC:2 * TC, 512:1024])
            xts[1] = xT1

            # Startup warm block: ~30 no-op matmuls keep the PE busy from
            # t~0.5us until the first xT/wT DMAs land, so the first
            # projection matmuls are priced at a ramped clock.
            warm0 = qpool.tile([128, 512], F32, tag='ctx', bufs=2,
                               name='warm0')
            for i in range(NWARM0):
                nc.tensor.matmul(warm0[:], ones_r[0:1, 0:128], ones_r[0:1, :],
                                 start=(i == 0), stop=(i == NWARM0 - 1))

            # v1 ones columns (col 64 of each 65-block), one strided memset
            ones_dst = bass.AP(v1.tensor, v1.offset + 64,
                               [list(v1.ap[0]), [65, 2 * NTT]])
            nc.vector.memset(ones_dst, 1.0)

            def proj_ops(ch):
                """QKV projections for chunk ch into qT/kT/v1, as a
                generator yielding every ~2 PE matmuls so attention can
                interleave this work into its exp-wait slots."""
                xT = xts.pop(ch)
                for w_t, b_c, dst in ((wqT, bq_col, qT), (wkT, bk_col, kT)):
                    ps = qpool.tile([128, 512], F32, tag='work', bufs=WORKBUFS, name='work')
                    for ht in range(HT):
                        nc.tensor.matmul(
                            ps[:], w_t[:, ht, :], xT[:, ht, :],
                            start=(ht == 0), stop=(ht == HT - 1))
                        if ht % YMOD == YMOD - 1:
                            yield
                    nc.vector.tensor_scalar_add(
                        dst[:, TC * ch:TC * (ch + 1)], ps[:], b_c[:])
                    yield
                # V in natural [tok, chan] layout: 4 col-slice accum
                # groups in one PSUM bank, rank-1 f32r bias
                ps = qpool.tile([128, 512], F32, tag='work', bufs=WORKBUFS, name='work')
                for tt in range(4):
                    cs = slice(128 * tt, 128 * (tt + 1))
                    for ht in range(HT):
                        nc.tensor.matmul(
                            ps[:, cs], xT[:, ht, cs], wvT[:, ht, :],
                            start=(ht == 0), stop=False)
                        if ht % YMOD == YMOD - 1:
                            yield
                    nc.tensor.matmul(ps[:, cs], ones_r[0:1, 0:128],
                                     bv_r[0:1, :], start=False, stop=True)
                    kt = 4 * ch + tt
                    base = 130 * kt
                    # [V_h0 | gap | V_h1]: one strided copy fills cols
                    # base..base+63 and base+65..base+128
                    dst = bass.AP(v1.tensor, v1.offset + base,
                                  [list(v1.ap[0]), [65, 2], [1, 64]])
                    nc.vector.tensor_copy(
                        dst, ps[:, cs].rearrange('p (g c) -> p g c', g=2))
                    yield

            # ---- L1: QKV + head-0 attention for all chunks, plus the first
            # MOVE chunks of head-1 (their exps fill L1's idle ACT time;
            # this shortens the ACT-bound L2 phase so X0 still hides) ----
            filler = _Filler()
            if 'L' in phases:
                for _ in proj_ops(0):
                    pass
                if NWARM1:
                    # small warm block: fills the PE-idle window while the
                    # chunk-1 xT DMA-transpose is still in flight
                    w1 = qpool.tile([128, 512], F32, tag='ctx', bufs=2,
                                    name='warm1')
                    for i in range(NWARM1):
                        nc.tensor.matmul(w1[:], ones_r[0:1, 0:128],
                                         ones_r[0:1, :],
                                         start=(i == 0),
                                         stop=(i == NWARM1 - 1))
                for ch in range(NCHUNK):
                    if ch + 2 < NCHUNK:
                        xts[ch + 2] = load_xt(ch + 2)
                    if ch + 1 < NCHUNK:
                        filler.it = proj_ops(ch + 1)
                    _attention(nc, sc, qpool, qT, kT, v1, ones_r, cmask, ident_b, ut01,
                               a2a_in0, ch, 0, filler=filler)
                    if ch < MOVE:
                        _attention(nc, sc, qpool, qT, kT, v1, ones_r, cmask, ident_b, ut01,
                                   a2a_in1, ch, 1, filler=filler)
                    filler.drain()

                # ---- X0: AllToAll for head 0 (overlaps L2) ----
                nc.gpsimd.collective_compute(
                    'AllToAll', mybir.AluOpType.bypass,
                    replica_groups=[list(range(NC))],
                    ins=[a2a_in0[:]], outs=[a2a_out0[:]],
                )

            # ---- W: Wo transpose on the DMA engines (runs during L2) ----
            if 'W' in phases:
                nc.sync.dma_start_transpose(woT[:], wo_d[:])

            ctxa = pp.tile([128, NC * TC], BF16, tag='ctxa', name='ctxa')

            # ---- L2: remaining head-1 attention ----
            if 'L' in phases:
                for ch in range(MOVE, NCHUNK):
                    if ch == NCHUNK - 1:
                        # h0 half of ctxa: ONE 3D-AP DMA, emitted before X1
                        # (so it waits only on X0) and before the last
                        # chunk's store dispatch (its SP-SEQ hold then
                        # overlaps the final attention, not the X1 launch).
                        nc.sync.dma_start(
                            ctxa[0:64, :].rearrange('p (i t) -> p i t', i=NC),
                            a2a_out0[:].rearrange('i p t -> p i t'))
                    _attention(nc, sc, qpool, qT, kT, v1, ones_r, cmask, ident_b, ut01,
                               a2a_in1, ch, 1, use_pb=False, ahead=AHEADL2)
                nc.gpsimd.collective_compute(
                    'AllToAll', mybir.AluOpType.bypass,
                    replica_groups=[list(range(NC))],
                    ins=[a2a_in1[:]], outs=[a2a_out1[:]],
                )
                # Warm-keeper: one long PE accumulation of no-op rank-1
                # matmuls into a scratch bank. It has no dependencies, so it
                # runs back-to-back from the moment L2's PE work ends until
                # roughly when X1 + the h1 ctxa DMA complete. This keeps the
                # tensor engine continuously busy (and at full p-state)
                # through the collective, so the E matmuls right after are
                # executed (and priced) at peak clock instead of cold.
                warm = qpool.tile([128, 512], F32, tag='ctx', bufs=2,
                                  name='warm')
                for i in range(NWARM):
                    nc.tensor.matmul(warm[:], ones_r[0:1, 0:128],
                                     ones_r[0:1, :],
                                     start=(i == 0), stop=(i == NWARM - 1))

            # ---- E: output projection for my 512 tokens ----
            if 'E' in phases:
                # two balanced halves: E's it=0..3 accumulation runs while
                # blocks 4..7 are still transferring
                half = NC // 2 * TC
                nc.sync.dma_start(
                    ctxa[64:128, 0:half].rearrange('p (i t) -> p i t',
                                                   i=NC // 2),
                    a2a_out1[0:NC // 2].rearrange('i p t -> p i t'))
                nc.sync.dma_start(
                    ctxa[64:128, half:].rearrange('p (i t) -> p i t',
                                                  i=NC // 2),
                    a2a_out1[NC // 2:].rearrange('i p t -> p i t'))
                for tt in range(4):
                    for oc in range(2):
                        ps = qpool.tile([128, 512], F32, tag='st', bufs=STBUFS, name='st')[:]
                        for it in range(NC):
                            nc.tensor.matmul(
                                ps[:],
                                ctxa[:, TC * it + 128 * tt:TC * it + 128 * (tt + 1)],
                                woT[:, it, 512 * oc:512 * (oc + 1)],
                                start=(it == 0), stop=False)
                        nc.tensor.matmul(ps[:], ones_r[0:1, 0:128],
                                         bo_r[0:1, 512 * oc:512 * (oc + 1)],
                                         start=False, stop=True)
                        o_sb = sc.tile([128, 512], F32, tag='o_sb', bufs=2, name='o_sb')
                        # alternate copy engines so consecutive tiles'
                        # PSUM->SBUF copies don't queue behind each other
                        if (2 * tt + oc) % 2 == 0:
                            nc.scalar.copy(o_sb[:], ps)
                        else:
                            nc.vector.tensor_copy(o_sb[:], ps)
                        nc.sync.dma_start(
                            out_d[128 * tt:128 * (tt + 1),
                                  512 * oc:512 * (oc + 1)], o_sb[:])

    nc.compile()
    _cache[key] = nc
    return nc


def kernel(hidden_states, Wq, bq, Wk, bk, Wv, bv, Wo, bo, **run_kwargs):
    nc = _build()
    bf = ml_dtypes.bfloat16
    hs = np.ascontiguousarray(
        np.asarray(hidden_states, np.float32).reshape(T, H)).astype(bf)
    Wq, Wk, Wv, Wo = (np.asarray(w, np.float32).astype(bf)
                      for w in (Wq, Wk, Wv, Wo))
    bq, bk, bv, bo = (np.asarray(b, np.float32) for b in (bq, bk, bv, bo))
    in_maps = []
    for c in range(NC):
        r = slice(128 * c, 128 * (c + 1))
        in_maps.append({
            'hs': hs,
            'wq': np.ascontiguousarray(Wq[r]),
            'wk': np.ascontiguousarray(Wk[r]),
            'wv': np.ascontiguousarray(Wv[r]),
            'wo': Wo,
            'bq': np.ascontiguousarray(bq[r].reshape(128, 1)),
            'bk': np.ascontiguousarray(bk[r].reshape(128, 1)),
            'bv': np.ascontiguousarray(bv[r].reshape(1, 128)),
            'bo': np.ascontiguousarray(bo.reshape(1, H)),
        })
    res = run_bass_kernel_spmd(nc, in_maps, core_ids=list(range(NC)), **run_kwargs)
    out = np.concatenate([res.results[c]['out'] for c in range(NC)], axis=0)
    kernel.last_results = res
    return out.reshape(B, S, H)
